# revision 1
# baseline (speedup 1.0000x reference)
"""BeatPooling segment-mean kernel for 8 Trainium2 NeuronCores.

Reference computation (per batch row):
    s = clip(bounds[:, 0], 0, T-1); e = max(s+1, min(bounds[:, 1], T))
    mean[m] = sum(frame[s_m:e_m]) / (e_m - s_m)
    out = concat([mean, fourier(pos)], -1) @ W + b         # [M, D]

Sharding: data-parallel over B (one batch row per core).

Algorithm (per core), all matmuls, no gpsimd (ap_gather costs ~30 ns per
index on the Q7 cores -- ~31 us per 1024 indices -- so every
gather-based formulation loses):

  1. Edge matmuls.  For each 128-frame block k, one f32r matmul with a
     host-built stationary operand uslots_k [128, 32]: column 0 is
     all-ones (the block sum), columns 1.. are inclusive prefix masks,
     one per distinct segment-boundary position (s-1 or e-1) falling in
     that block.  The moving operand is the frame tile [128, 512].  The
     PSUM result P'_k[slot, d] holds every within-block prefix the
     output needs.  f32r streams at 1 cycle/row, so the whole 16 MiB
     frame row costs ~64 x 0.3 us of PE time and is never transposed,
     scanned, or cast.
  2. P' tiles are evacuated to SBUF as fp16 (0.05% worst-case error --
     well within the 2e-2 gate).
  3. Combine matmuls.  segT[d, m] = sum_t pv_t^T . G_t, accumulated in
     PSUM over the 16 slot-tiles as they appear.  G_t [128 slots, 512 m]
     (host-built fp16, +-1 one-hots at each segment's e/s boundary
     slots) also absorbs the block-span part: its slot-0 rows carry the
     0/1 band J[k, m] = [K_s(m) <= k < K_e(m)], which multiplies the
     block sums.  So segT accumulates (P_e - P_s + sum of spanned block
     sums) == the full segment sums, transposed, ready for projection.
  4. Projection in fp16 (W1 host-packed), then one scalar_tensor_tensor
     fuses the 1/count scale (per-partition scalar) and the fourier/bias
     term (computed on device from a tiny packed tensor by one more
     matmul per m-tile).

DMA notes: all aux tensors ride in a few large contiguous DMAs (the
original baseline lost ~50 us draining dozens of tiny per-partition
descriptors), and the 16 MiB frame stream alternates between the two
HWDGE rings (sync/scalar) in 2 MiB chunks.
"""

import math

import numpy as np

import concourse.bacc as bacc
import concourse.mybir as mybir
from concourse import bass_utils
from concourse.tile import TileContext

B, T, D, M = 8, 8192, 512, 512
POS_DIM = 32
P = 128
N_CORES = 8
NB = T // P            # 64 blocks of 128 frames
GROUPS = 8             # stream groups (8 blocks = 2 MiB each)
BPG = NB // GROUPS     # blocks per group
DC = D // P            # 4 d-chunks
MC = M // P            # 4 m-chunks

F32 = mybir.dt.float32
F32R = mybir.dt.float32r
F16 = mybir.dt.float16
BF16 = mybir.dt.bfloat16

_CACHED_NC = {}


def _build_nc(S):
    NSLOT = NB * S
    TPB = P // S           # blocks per slot-tile (4 for S=32)
    NT = NB // TPB         # slot-tiles (16)

    nc = bacc.Bacc("TRN2", target_bir_lowering=False, debug=False,
                   num_devices=N_CORES)

    frame = nc.dram_tensor("frame", [T, D], F32R, kind="ExternalInput")
    us_in = nc.dram_tensor("uslots", [P, NSLOT + 4], F32R,
                           kind="ExternalInput")
    g_in = nc.dram_tensor("gmat", [P, NT * M], F16, kind="ExternalInput")
    w1_in = nc.dram_tensor("w1p", [P, DC * D], F16, kind="ExternalInput")
    ffw2_in = nc.dram_tensor("ffw2", [P, D], F32R, kind="ExternalInput")
    out = nc.dram_tensor("out", [M, D], F32, kind="ExternalOutput")

    add = mybir.AluOpType.add
    mult = mybir.AluOpType.mult

    with TileContext(nc, num_cores=N_CORES) as tc:
        with (
            tc.tile_pool(name="const", bufs=1) as const,
            tc.tile_pool(name="staging", bufs=4) as staging,
            tc.tile_pool(name="psum", bufs=4, space="PSUM") as psum,
            tc.tile_pool(name="pacc", bufs=1, space="PSUM") as pacc,
            tc.tile_pool(name="outp", bufs=2) as outp,
        ):
            # ---- long-lived tiles -------------------------------------
            uslots = const.tile([P, NSLOT + 4], F32R, name="uslots")
            gmat = const.tile([P, NT * M], F16, name="gmat")
            pvall = const.tile([P, NT * D], F16, name="pvall")
            w1t = const.tile([P, DC * D], F16, name="w1t")
            ffa = const.tile([64, D], F32R, name="ffa")
            ffb = const.tile([64, D], F32R, name="ffb")
            segsb = const.tile([P, DC * M], F16, name="segsb")
            outall = const.tile([P, MC * D], F32, name="outall")
            biassb = const.tile([P, MC * D], F32, name="biassb")

            recip_v = uslots[:, NSLOT:NSLOT + 4].bitcast(F32)

            # ---- constant DMAs (uslots gates the first edge matmul;
            # gmat/w1 are needed only later and ride mid/late on the
            # sync ring to balance ring bytes) ----
            UQ = (NSLOT + 4) // 4
            QW = NT * M // 4
            nc.sync.dma_start(uslots[:, 0:UQ], us_in.ap()[:, 0:UQ])
            nc.scalar.dma_start(gmat[:, 0:QW], g_in.ap()[:, 0:QW])

            # segT accumulators, one per d-chunk, live across the stream
            po = [pacc.tile([P, M], F32, name=f"po_{c}", tag=f"po{c}")
                  for c in range(DC)]

            # gmat arrives in just-in-time quarters on the scalar ring
            # so the combine matmuls never wait and the sync ring carries a
            # pure, uninterrupted frame stream

            # ---- stream frame ----------------------------------------
            frame_g = frame.ap().rearrange("(g b p) d -> g p b d", p=P, b=BPG)
            for g in range(GROUPS):
                st = staging.tile([P, BPG * D], F32R, name="stage",
                                  tag="stage")
                if g == 1:
                    nc.scalar.dma_start(uslots[:, UQ:2 * UQ],
                                        us_in.ap()[:, UQ:2 * UQ])
                if g in (1, 3, 5):
                    q = g // 2 + 1
                    nc.scalar.dma_start(gmat[:, q * QW:(q + 1) * QW],
                                        g_in.ap()[:, q * QW:(q + 1) * QW])
                if g == 2:
                    nc.scalar.dma_start(uslots[:, 2 * UQ:],
                                        us_in.ap()[:, 2 * UQ:])
                if g == 5:
                    nc.scalar.dma_start(ffa[:], ffw2_in.ap()[0:64, :])
                    nc.scalar.dma_start(ffb[:], ffw2_in.ap()[64:128, :])
                if g == 6:
                    nc.scalar.dma_start(w1t[:], w1_in.ap())
                stv = st[:].rearrange("p (b d) -> p b d", b=BPG)
                if g == 0:
                    nc.sync.dma_start(stv[:, 0:2], frame_g[g][:, 0:2])
                    nc.sync.dma_start(stv[:, 2:4], frame_g[g][:, 2:4])
                    nc.sync.dma_start(stv[:, 4:8], frame_g[g][:, 4:8])
                else:
                    nc.sync.dma_start(stv, frame_g[g])
                for b in range(BPG):
                    k = g * BPG + b
                    i = k % TPB
                    t = k // TPB
                    pp = psum.tile([S, D], F32, name=f"pp_{k}", tag="ps")
                    nc.tensor.matmul(
                        pp[:],
                        lhsT=uslots[:, k * S:(k + 1) * S],
                        rhs=st[:, b * D:(b + 1) * D],
                        start=True, stop=True,
                    )
                    if i % 2 == 0:
                        nc.vector.tensor_scalar_add(
                            out=pvall[i * S:(i + 1) * S,
                                      t * D:(t + 1) * D],
                            in0=pp[:], scalar1=0.0)
                    else:
                        nc.scalar.copy(
                            pvall[i * S:(i + 1) * S, t * D:(t + 1) * D],
                            pp[:])
                    if i == TPB - 1 and t % 4 == 3 and t < NT - 1:
                        # combine, batched 4 tiles at a time and grouped by
                        # chunk so each po[c] accumulation chain runs four
                        # consecutive steps (uninterrupted chains avoid a
                        # per-switch PE overhead)
                        for c in range(DC):
                            for tl in range(t - 3, t + 1):
                                nc.tensor.matmul(
                                    po[c][:],
                                    lhsT=pvall[:, tl * D + c * P:
                                               tl * D + (c + 1) * P],
                                    rhs=gmat[:, tl * M:(tl + 1) * M],
                                    start=(tl == 0), stop=False,
                                )

            # fourier/bias term: bias[m, j] = ff[m] @ W2 + b (PE slack
            # while the last stream group lands; needed only by the stt)
            for mt in range(MC):
                bps = psum.tile([P, D], F32, name=f"bps_{mt}", tag="ps")
                nc.tensor.matmul(
                    bps[:],
                    lhsT=ffa[:, mt * P:(mt + 1) * P],
                    rhs=ffb[:],
                    start=True, stop=True,
                )
                nc.scalar.copy(biassb[:, mt * D:(mt + 1) * D], bps[:])

            # last batch of combines
            for c in range(DC):
                for tl in range(NT - 4, NT):
                    nc.tensor.matmul(
                        po[c][:],
                        lhsT=pvall[:, tl * D + c * P:tl * D + (c + 1) * P],
                        rhs=gmat[:, tl * M:(tl + 1) * M],
                        start=False, stop=(tl == NT - 1),
                    )

            # ---- segT -> SBUF fp16, project, scale, bias --------------
            for c in range(DC):
                nc.vector.tensor_scalar_add(
                    out=segsb[:, c * M:(c + 1) * M], in0=po[c][:],
                    scalar1=0.0)
            for mt in range(MC):
                po2 = psum.tile([P, D], F32, name=f"po2_{mt}", tag="ps")
                for c in range(DC):
                    nc.tensor.matmul(
                        po2[:],
                        lhsT=segsb[:, c * M + mt * P:c * M + (mt + 1) * P],
                        rhs=w1t[:, c * D:(c + 1) * D],
                        start=(c == 0), stop=(c == DC - 1),
                    )
                nc.vector.scalar_tensor_tensor(
                    out=outall[:, mt * D:(mt + 1) * D],
                    in0=po2[:],
                    scalar=recip_v[:, mt:mt + 1],
                    in1=biassb[:, mt * D:(mt + 1) * D],
                    op0=mult,
                    op1=add,
                )
            outv = out.ap().rearrange("(mt p) d -> p mt d", p=P)
            oall = outall[:].rearrange("p (mt d) -> p mt d", mt=MC)
            nc.sync.dma_start(outv[:, 0:2], oall[:, 0:2])
            nc.scalar.dma_start(outv[:, 2:4], oall[:, 2:4])

    nc.compile()
    return nc


def _fourier_features(pos, dim):
    half = dim // 2
    freqs = np.exp(np.linspace(0.0, math.log(1000.0), half))
    ang = pos[..., None] * freqs
    return np.concatenate([np.sin(ang), np.cos(ang)], axis=-1)


def _host_prep(frame_emb, beat_bounds, W, b, S):
    NSLOT = NB * S
    TPB = P // S
    NT = NB // TPB

    s_all = np.clip(beat_bounds[:, :, 0], 0, T - 1).astype(np.int64)
    e_all = np.maximum(
        s_all + 1, np.minimum(beat_bounds[:, :, 1], T)).astype(np.int64)
    recip_all = (1.0 / (e_all - s_all)).astype(np.float32)

    pos = np.clip(np.arange(M, dtype=np.float64) / max(1, M - 1), 0.0, 1.0)
    ff = _fourier_features(pos, POS_DIM)                  # [M, 32]
    # rows 0:64 = [ff^T; ones; pad] (cols = m), rows 64:128 = [W2; b; pad]
    ffw2 = np.zeros((P, D), dtype=np.float32)
    ffw2[0:POS_DIM, :] = ff.T.astype(np.float32)
    ffw2[POS_DIM, :] = 1.0
    ffw2[64:64 + POS_DIM, :] = W[D:D + POS_DIM, :].astype(np.float32)
    ffw2[64 + POS_DIM, :] = b.astype(np.float32)

    w1p = np.ascontiguousarray(
        W[:D, :].astype(np.float16).reshape(DC, P, D)
        .transpose(1, 0, 2).reshape(P, DC * D))

    # U[p, o] = 1.0 if p <= o (inclusive prefix-mask columns)
    U = (np.arange(P)[:, None] <= np.arange(P)[None, :]).astype(np.float32)

    in_maps = []
    for i in range(B):
        s, e = s_all[i], e_all[i]
        allpos = np.concatenate([(s - 1)[s > 0], e - 1])
        uslots = np.zeros((P, NSLOT + 4), dtype=np.float32)
        slotmap = {}
        for k in range(NB):
            offs = np.unique(allpos[(allpos >> 7) == k] & 127)
            if len(offs) > S - 1:
                raise OverflowError(
                    f"block {k}: {len(offs)} boundaries > {S - 1}")
            uslots[:, k * S] = 1.0                         # block-sum slot
            for j, o in enumerate(offs):
                uslots[:, k * S + 1 + j] = U[:, o]
                slotmap[(k, int(o))] = k * S + 1 + j
        uslots[:, NSLOT:NSLOT + 4] = recip_all[i].reshape(MC, P).T

        # G_t[slot, m]: +1 at e-boundary slot, -1 at s-boundary slot,
        # 0/1 block-span band J on the slot-0 rows
        gm = np.zeros((NT, P, M), dtype=np.float32)
        for m in range(M):
            pe = int(e[m]) - 1
            ke = pe >> 7
            sl = slotmap[(ke, pe & 127)]
            gm[sl // P, sl % P, m] += 1.0
            ks = 0
            if s[m] > 0:
                ps = int(s[m]) - 1
                ks = ps >> 7
                sl = slotmap[(ks, ps & 127)]
                gm[sl // P, sl % P, m] -= 1.0
            for k in range(ks, ke):
                sl = k * S
                gm[sl // P, sl % P, m] += 1.0
        gmat = np.ascontiguousarray(
            gm.transpose(1, 0, 2).reshape(P, NT * M)).astype(np.float16)

        in_maps.append({
            "frame": np.ascontiguousarray(frame_emb[i], dtype=np.float32),
            "uslots": uslots,
            "gmat": gmat,
            "w1p": w1p,
            "ffw2": ffw2,
        })
    return in_maps


def get_nc(S=32):
    if S not in _CACHED_NC:
        _CACHED_NC[S] = _build_nc(S)
    return _CACHED_NC[S]


def kernel(frame_emb, beat_bounds, W, b, _trace=False):
    frame_emb = np.asarray(frame_emb)
    beat_bounds = np.asarray(beat_bounds)
    W = np.asarray(W)
    b = np.asarray(b)
    in_maps = None
    for S in (32, 64):
        try:
            in_maps = _host_prep(frame_emb, beat_bounds, W, b, S)
            break
        except OverflowError:
            continue
    if in_maps is None:
        raise RuntimeError("too many segment boundaries per 128-frame block")
    nc = get_nc(S)
    res = bass_utils.run_bass_kernel_spmd(
        nc, in_maps, core_ids=list(range(N_CORES)), trace=_trace)
    out = np.stack([res.results[i]["out"] for i in range(B)], axis=0)
    if _trace:
        kernel.last_results = res
    return out



# revision 17
# speedup vs baseline: 1.2727x; 1.2727x over previous
"""BeatPooling segment-mean kernel for 8 Trainium2 NeuronCores.

Reference computation (per batch row):
    s = clip(bounds[:, 0], 0, T-1); e = max(s+1, min(bounds[:, 1], T))
    mean[m] = sum(frame[s_m:e_m]) / (e_m - s_m)
    out = concat([mean, fourier(pos)], -1) @ W + b         # [M, D]

Sharding: data-parallel over B (one batch row per core).

Algorithm (per core).  The whole thing is matmuls; no gpsimd.

  1. The frame row is pre-cast to fp16 on the host (halves the HBM
     stream; rel-err stays ~6e-3 vs the 2e-2 gate) and laid out so each
     SBUF partition receives 4 *consecutive* frame rows = one contiguous
     4 KiB DMA descriptor (vs the 2 KiB descriptors a plain
     frame-per-partition layout forces; the DMA engines are
     per-descriptor-throughput-bound, not HBM-bound).
  2. Edge matmuls.  Per 512-frame superblock k: 4 accumulating matmuls
     (one per within-partition sub-row j) with host-built fp16
     stationary masks U_j[p, slot] = [4p + j <= off(slot)], streamed in
     just-in-time quarters.  Slot 0 of each superblock is the all-ones
     column (the block sum); slots 1.. are the distinct
     segment-boundary positions (s-1 / e-1) falling in that superblock.
     PSUM result pp[slot, d] holds every within-block prefix the output
     needs; evicted to SBUF as fp16 (pvall), one clean [128, 512] tile
     per superblock.
  3. Combine matmuls.  segT[d, m] = sum_t pvall_t^T . G_t accumulated in
     PSUM over the 16 slot-tiles.  G_t [128 slots, 512 m] (host-built
     fp16) carries +recip[m] at each segment's e-boundary slot,
     -recip[m] at the s-boundary slot, and +recip[m] on the slot-0 rows
     of every fully-spanned superblock - so segT accumulates the segment
     *means* directly (recip folded in; no separate scale pass).
  4. Projection: 4 d-chunk matmuls with fp16 W1 plus a 5th 64-deep
     chunk [ff^T; ones] @ [W2; b] that adds the fourier/bias term inside
     the same PSUM accumulation.  The result only needs a cast-copy to
     fp16 and a 2-ring DMA out; the host upcasts to f32.
"""

import math

import numpy as np

import concourse.bacc as bacc
import concourse.mybir as mybir
from concourse import bass_utils
from concourse.tile import TileContext

B, T, D, M = 8, 8192, 512, 512
POS_DIM = 32
P = 128
N_CORES = 8
S = 128                # slots per superblock (= one combine tile)
DC = D // P            # 4 d-chunks
MC = M // P            # 4 m-chunks

F32 = mybir.dt.float32
F16 = mybir.dt.float16

_CACHED_NC = {}


def _build_nc(SB):
    JS = SB // P           # consecutive frame rows per partition
    NSB = T // SB          # superblocks == combine tiles
    NT = NSB
    NIDX = NSB * JS * S    # mask columns
    NAUX = 1024            # ffpack | w2pack

    nc = bacc.Bacc("TRN2", target_bir_lowering=False, debug=False,
                   num_devices=N_CORES)

    frame = nc.dram_tensor("frame", [T, D], F16, kind="ExternalInput")
    us_in = nc.dram_tensor("uslots", [P, NIDX], F16, kind="ExternalInput")
    g_in = nc.dram_tensor("gmat", [P, NT * M], F16, kind="ExternalInput")
    w1_in = nc.dram_tensor("w1p", [P, DC * D], F16, kind="ExternalInput")
    aux_in = nc.dram_tensor("aux", [P, NAUX], F16, kind="ExternalInput")
    out = nc.dram_tensor("out", [M, D], F16, kind="ExternalOutput")

    with TileContext(nc, num_cores=N_CORES) as tc:
        with (
            tc.tile_pool(name="const", bufs=1) as const,
            tc.tile_pool(name="staging", bufs=8) as staging,
            tc.tile_pool(name="psum", bufs=4, space="PSUM") as psum,
            tc.tile_pool(name="pacc", bufs=1, space="PSUM") as pacc,
        ):
            # ---- long-lived tiles -------------------------------------
            uslots = const.tile([P, NIDX], F16, name="uslots")
            gmat = const.tile([P, NT * M], F16, name="gmat")
            pvall = const.tile([P, NT * D], F16, name="pvall")
            w1t = const.tile([P, DC * D], F16, name="w1t")
            aux = const.tile([P, NAUX], F16, name="aux")
            segsb = const.tile([P, DC * M], F16, name="segsb")
            outall = const.tile([P, MC * D], F16, name="outall")

            # uslots quarter 0 gates the first edge matmul and rides the
            # scalar ring first; w1t/aux are small and load right away so
            # the projection tail never waits on them; the remaining
            # uslots/gmat quarters arrive just-in-time ahead of their
            # consumers.
            UQ = NIDX // 4
            GQ = NT * M // 4
            nc.scalar.dma_start(uslots[:, 0:UQ], us_in.ap()[:, 0:UQ])
            nc.scalar.dma_start(w1t[:], w1_in.ap())
            nc.scalar.dma_start(aux[:], aux_in.ap())

            # segT accumulators, one per d-chunk, live across the stream
            po = [pacc.tile([P, M], F32, name=f"po_{c}", tag=f"po{c}")
                  for c in range(DC)]

            ku = {max(1, q * NSB // 4 - 3): q for q in (1, 2, 3)}
            kq = {max(0, q * NSB // 4 - 2): q for q in (1, 2, 3)}

            frame_g = frame.ap().rearrange("(k p j) d -> k p (j d)",
                                           p=P, j=JS)

            # ---- stream frame ----------------------------------------
            # Frame superblocks alternate between the sync HWDGE queue
            # and the gpsimd SWDGE queue so one queue's staging-buffer
            # wait never head-of-line-blocks the next transfer.  The
            # combine for tile t is issued after edge t+1 so the PE never
            # stalls on tile t's PSUM->SBUF eviction latency.
            def combine(t, stop):
                for c in range(DC):
                    nc.tensor.matmul(
                        po[c][:],
                        lhsT=pvall[:, t * D + c * P:t * D + (c + 1) * P],
                        rhs=gmat[:, t * M:(t + 1) * M],
                        start=(t == 0), stop=stop,
                    )

            for k in range(NSB):
                if k == 0:
                    nc.scalar.dma_start(gmat[:, 0:GQ], g_in.ap()[:, 0:GQ])
                if k in ku:
                    q = ku[k]
                    nc.scalar.dma_start(uslots[:, q * UQ:(q + 1) * UQ],
                                        us_in.ap()[:, q * UQ:(q + 1) * UQ])
                if k in kq:
                    q = kq[k]
                    nc.scalar.dma_start(gmat[:, q * GQ:(q + 1) * GQ],
                                        g_in.ap()[:, q * GQ:(q + 1) * GQ])

                st = staging.tile([P, JS * D], F16, name="stage",
                                  tag="stage")
                ring = nc.sync if k % 2 == 0 else nc.gpsimd
                ring.dma_start(st[:], frame_g[k])

                pp = psum.tile([S, D], F32, name=f"pp_{k}", tag="ps")
                for jj in range(JS):
                    nc.tensor.matmul(
                        pp[:],
                        lhsT=uslots[:, (k * JS + jj) * S:
                                    (k * JS + jj + 1) * S],
                        rhs=st[:, jj * D:(jj + 1) * D],
                        start=(jj == 0), stop=(jj == JS - 1),
                    )
                if k % 2 == 0:
                    nc.vector.tensor_scalar_add(
                        out=pvall[:, k * D:(k + 1) * D], in0=pp[:],
                        scalar1=0.0)
                else:
                    nc.scalar.copy(pvall[:, k * D:(k + 1) * D], pp[:])

                if k >= 1:
                    combine(k - 1, stop=False)

            # ---- tail: last combine, evict, project, cast, store ------
            # Per d-chunk: close the po[c] accumulation with tile NT-1,
            # evict it, and let its projection matmuls run while the next
            # chunk combines - keeps the PE busy through the whole tail.
            po2 = [psum.tile([P, D], F32, name=f"po2_{mt}", tag="ps")
                   for mt in range(MC)]
            for c in range(DC):
                t = NT - 1
                nc.tensor.matmul(
                    po[c][:],
                    lhsT=pvall[:, t * D + c * P:t * D + (c + 1) * P],
                    rhs=gmat[:, t * M:(t + 1) * M],
                    start=False, stop=True,
                )
                if c % 2 == 0:
                    nc.vector.tensor_scalar_add(
                        out=segsb[:, c * M:(c + 1) * M], in0=po[c][:],
                        scalar1=0.0)
                else:
                    nc.scalar.copy(segsb[:, c * M:(c + 1) * M], po[c][:])
                for mt in range(MC):
                    nc.tensor.matmul(
                        po2[mt][:],
                        lhsT=segsb[:, c * M + mt * P:c * M + (mt + 1) * P],
                        rhs=w1t[:, c * D:(c + 1) * D],
                        start=(c == 0), stop=False,
                    )
            outv = out.ap().rearrange("(mt p) d -> p mt d", p=P)
            oall = outall[:].rearrange("p (mt d) -> p mt d", mt=MC)
            rings = [nc.sync, nc.scalar, nc.gpsimd, nc.scalar]
            for mt in range(MC):
                nc.tensor.matmul(
                    po2[mt][:],
                    lhsT=aux[0:64, mt * P:(mt + 1) * P],
                    rhs=aux[0:64, 512:1024],
                    start=False, stop=True,
                )
                if mt % 2 == 0:
                    nc.vector.tensor_scalar_add(
                        out=outall[:, mt * D:(mt + 1) * D], in0=po2[mt][:],
                        scalar1=0.0)
                else:
                    nc.scalar.copy(outall[:, mt * D:(mt + 1) * D],
                                   po2[mt][:])
                rings[mt].dma_start(outv[:, mt:mt + 1], oall[:, mt:mt + 1])

    nc.compile()
    return nc


def _fourier_features(pos, dim):
    half = dim // 2
    freqs = np.exp(np.linspace(0.0, math.log(1000.0), half))
    ang = pos[..., None] * freqs
    return np.concatenate([np.sin(ang), np.cos(ang)], axis=-1)


def _host_prep(frame_emb, beat_bounds, W, b, SB):
    JS = SB // P
    NSB = T // SB
    NT = NSB
    NIDX = NSB * JS * S

    s_all = np.clip(beat_bounds[:, :, 0], 0, T - 1).astype(np.int64)
    e_all = np.maximum(
        s_all + 1, np.minimum(beat_bounds[:, :, 1], T)).astype(np.int64)
    recip_all = (1.0 / (e_all - s_all)).astype(np.float32)

    pos = np.clip(np.arange(M, dtype=np.float64) / max(1, M - 1), 0.0, 1.0)
    ff = _fourier_features(pos, POS_DIM)                  # [M, 32]
    aux = np.zeros((P, 1024), dtype=np.float16)
    aux[0:POS_DIM, 0:M] = ff.T.astype(np.float16)
    aux[POS_DIM, 0:M] = 1.0
    aux[0:POS_DIM, 512:512 + D] = W[D:D + POS_DIM, :].astype(np.float16)
    aux[POS_DIM, 512:512 + D] = b.astype(np.float16)

    w1p = np.ascontiguousarray(
        W[:D, :].astype(np.float16).reshape(DC, P, D)
        .transpose(1, 0, 2).reshape(P, DC * D))

    in_maps = []
    for i in range(B):
        s, e, recip = s_all[i], e_all[i], recip_all[i]
        allpos = np.concatenate([(s - 1)[s > 0], e - 1])
        idxv = np.full(NIDX, -1.0, dtype=np.float32)
        slotmap = {}
        for k in range(NSB):
            offs = np.unique(allpos[(allpos // SB) == k] % SB)
            if len(offs) > S - 1:
                raise OverflowError(
                    f"superblock {k}: {len(offs)} boundaries > {S - 1}")
            base = k * JS * S
            for jj in range(JS):
                idxv[base + jj * S] = (SB - 1 - jj) // JS   # sum slot
                idxv[base + jj * S + 1:base + jj * S + 1 + len(offs)] = (
                    np.where(offs >= jj, (offs - jj) // JS, -1.0))
            for j, o in enumerate(offs):
                slotmap[(k, int(o))] = j + 1

        # stationary masks: uslots[p, c] = (idxv[c] >= p)
        us = (idxv[None, :] >= np.arange(P, dtype=np.float32)[:, None])

        # G[slot, m]: +r at e-boundary slot, -r at s-boundary slot,
        # +r on the sum slots of fully-spanned superblocks
        gm = np.zeros((NSB * S, M), dtype=np.float32)
        for m in range(M):
            r = recip[m]
            pe = int(e[m]) - 1
            ke = pe // SB
            gm[ke * S + slotmap[(ke, pe % SB)], m] += r
            ks = 0
            if s[m] > 0:
                ps = int(s[m]) - 1
                ks = ps // SB
                gm[ks * S + slotmap[(ks, ps % SB)], m] -= r
            for k in range(ks, ke):
                gm[k * S, m] += r
        gmat = np.ascontiguousarray(
            gm.reshape(NT, P, M).transpose(1, 0, 2)
            .reshape(P, NT * M)).astype(np.float16)

        in_maps.append({
            "frame": np.ascontiguousarray(frame_emb[i], dtype=np.float16),
            "uslots": us.astype(np.float16),
            "gmat": gmat,
            "w1p": w1p,
            "aux": aux,
        })
    return in_maps


def get_nc(SB=512):
    if SB not in _CACHED_NC:
        _CACHED_NC[SB] = _build_nc(SB)
    return _CACHED_NC[SB]


def kernel(frame_emb, beat_bounds, W, b, _trace=False):
    frame_emb = np.asarray(frame_emb)
    beat_bounds = np.asarray(beat_bounds)
    W = np.asarray(W)
    b = np.asarray(b)
    in_maps = None
    for SB in (512, 256, 128):
        try:
            in_maps = _host_prep(frame_emb, beat_bounds, W, b, SB)
            break
        except OverflowError:
            continue
    if in_maps is None:
        raise RuntimeError("too many segment boundaries per superblock")
    nc = get_nc(SB)
    res = bass_utils.run_bass_kernel_spmd(
        nc, in_maps, core_ids=list(range(N_CORES)), trace=_trace)
    out = np.stack([res.results[i]["out"] for i in range(B)],
                   axis=0).astype(np.float32)
    if _trace:
        kernel.last_results = res
    return out


# revision 26
# speedup vs baseline: 1.5319x; 1.2036x over previous
"""BeatPooling segment-mean kernel for 8 Trainium2 NeuronCores.

Reference computation (per batch row):
    s = clip(bounds[:, 0], 0, T-1); e = max(s+1, min(bounds[:, 1], T))
    mean[m] = sum(frame[s_m:e_m]) / (e_m - s_m)
    out = concat([mean, fourier(pos)], -1) @ W + b         # [M, D]

Sharding: data-parallel over B (one batch row per core).

Algorithm (per core).  The whole thing is matmuls; no gpsimd.

  1. The frame row is pre-cast to fp16 on the host (halves the HBM
     stream; rel-err stays ~6e-3 vs the 2e-2 gate) and laid out so each
     SBUF partition receives 4 *consecutive* frame rows = one contiguous
     4 KiB DMA descriptor (vs the 2 KiB descriptors a plain
     frame-per-partition layout forces; the DMA engines are
     per-descriptor-throughput-bound, not HBM-bound).
  2. Edge matmuls.  Per 512-frame superblock k: 4 accumulating matmuls
     (one per within-partition sub-row j) with host-built fp16
     stationary masks U_j[p, slot] = [4p + j <= off(slot)], streamed in
     just-in-time quarters.  Slot 0 of each superblock is the all-ones
     column (the block sum); slots 1.. are the distinct
     segment-boundary positions (s-1 / e-1) falling in that superblock.
     PSUM result pp[slot, d] holds every within-block prefix the output
     needs; evicted to SBUF as fp16 (pvall), one clean [128, 512] tile
     per superblock.
  3. Combine matmuls.  segT[d, m] = sum_t pvall_t^T . G_t accumulated in
     PSUM over the 16 slot-tiles.  G_t [128 slots, 512 m] (host-built
     fp16) carries +recip[m] at each segment's e-boundary slot,
     -recip[m] at the s-boundary slot, and +recip[m] on the slot-0 rows
     of every fully-spanned superblock - so segT accumulates the segment
     *means* directly (recip folded in; no separate scale pass).
  4. Projection: 4 d-chunk matmuls with fp16 W1 plus a 5th 64-deep
     chunk [ff^T; ones] @ [W2; b] that adds the fourier/bias term inside
     the same PSUM accumulation.  The result only needs a cast-copy to
     fp16 and a 2-ring DMA out; the host upcasts to f32.
"""

import math

import numpy as np

import concourse.bacc as bacc
import concourse.mybir as mybir
from concourse import bass_utils
from concourse.tile import TileContext

B, T, D, M = 8, 8192, 512, 512
POS_DIM = 32
P = 128
N_CORES = 8
S = 128                # slots per superblock (= one combine tile)
DC = D // P            # 4 d-chunks
MC = M // P            # 4 m-chunks

F32 = mybir.dt.float32
F16 = mybir.dt.float16
F8 = mybir.dt.float8e4

_CACHED_NC = {}


def _build_nc(SB):
    JS = SB // P           # consecutive frame rows per partition
    NSB = T // SB          # superblocks == combine tiles
    NT = NSB
    NIDX = NSB * JS * S    # mask columns
    NAUX = 1032            # ffcnt | w2pack | recip (f32 as 2xf16)

    nc = bacc.Bacc("TRN2", target_bir_lowering=False, debug=False,
                   num_devices=N_CORES)

    frame = nc.dram_tensor("frame", [T, D], F16, kind="ExternalInput")
    us_in = nc.dram_tensor("uslots", [P, NIDX], F8, kind="ExternalInput")
    g_in = nc.dram_tensor("gmat", [P, NT * M], F8, kind="ExternalInput")
    w1_in = nc.dram_tensor("w1p", [P, DC * D], F16, kind="ExternalInput")
    aux_in = nc.dram_tensor("aux", [P, NAUX], F16, kind="ExternalInput")
    out = nc.dram_tensor("out", [M, D], F16, kind="ExternalOutput")

    mult = mybir.AluOpType.mult

    with TileContext(nc, num_cores=N_CORES) as tc:
        with (
            tc.tile_pool(name="const", bufs=1) as const,
            tc.tile_pool(name="staging", bufs=8) as staging,
            tc.tile_pool(name="psum", bufs=4, space="PSUM") as psum,
            tc.tile_pool(name="pacc", bufs=1, space="PSUM") as pacc,
        ):
            # ---- long-lived tiles -------------------------------------
            uslots = const.tile([P, NIDX], F8, name="uslots")
            gmat = const.tile([P, NT * M], F8, name="gmat")
            pvall = const.tile([P, NT * D], F16, name="pvall")
            w1t = const.tile([P, DC * D], F16, name="w1t")
            aux = const.tile([P, NAUX], F16, name="aux")
            segsb = const.tile([P, DC * M], F16, name="segsb")
            outall = const.tile([P, MC * D], F16, name="outall")

            # uslots quarter 0 gates the first edge matmul and rides the
            # scalar ring first; w1t/aux are small and load right away so
            # the projection tail never waits on them; the remaining
            # uslots/gmat quarters arrive just-in-time ahead of their
            # consumers.
            UQ = NIDX // 4
            GQ = NT * M // 4
            nc.scalar.dma_start(uslots[:, 0:UQ], us_in.ap()[:, 0:UQ])
            nc.scalar.dma_start(w1t[:], w1_in.ap())
            nc.scalar.dma_start(aux[:], aux_in.ap())

            # segT accumulators, one per d-chunk, live across the stream
            po = [pacc.tile([P, M], F32, name=f"po_{c}", tag=f"po{c}")
                  for c in range(DC)]

            ku = {max(1, q * NSB // 4 - 3): q for q in (1, 2, 3)}
            kq = {max(0, q * NSB // 4 - 2): q for q in (1, 2, 3)}

            frame_g = frame.ap().rearrange("(k p j) d -> k p (j d)",
                                           p=P, j=JS)

            # ---- stream frame ----------------------------------------
            # The combine for tile t is issued after edge t+2, giving
            # tile t's PSUM->SBUF eviction two superblocks (~3 us) of
            # slack - the PE then runs edge and combine back-to-back with
            # no semaphore joins, stays continuously busy, and ramps to
            # its full clock.
            def combine(t, stop):
                for c in range(DC):
                    nc.tensor.matmul(
                        po[c][:],
                        lhsT=pvall[:, t * D + c * P:t * D + (c + 1) * P],
                        rhs=gmat[:, t * M:(t + 1) * M],
                        start=(t == 0), stop=stop,
                    )

            for k in range(NSB):
                if k == 0:
                    nc.scalar.dma_start(gmat[:, 0:GQ], g_in.ap()[:, 0:GQ])
                if k in ku:
                    q = ku[k]
                    nc.scalar.dma_start(uslots[:, q * UQ:(q + 1) * UQ],
                                        us_in.ap()[:, q * UQ:(q + 1) * UQ])
                if k in kq:
                    q = kq[k]
                    nc.scalar.dma_start(gmat[:, q * GQ:(q + 1) * GQ],
                                        g_in.ap()[:, q * GQ:(q + 1) * GQ])

                st = staging.tile([P, JS * D], F16, name="stage",
                                  tag="stage")
                nc.sync.dma_start(st[:], frame_g[k])

                pp = psum.tile([S, D], F32, name=f"pp_{k}", tag="ps")
                for jj in range(JS):
                    nc.tensor.matmul(
                        pp[:],
                        lhsT=uslots[:, (k * JS + jj) * S:
                                    (k * JS + jj + 1) * S],
                        rhs=st[:, jj * D:(jj + 1) * D],
                        start=(jj == 0), stop=(jj == JS - 1),
                    )
                if k % 2 == 0:
                    nc.vector.tensor_scalar_add(
                        out=pvall[:, k * D:(k + 1) * D], in0=pp[:],
                        scalar1=0.0)
                else:
                    nc.scalar.copy(pvall[:, k * D:(k + 1) * D], pp[:])

                if k >= 2:
                    combine(k - 2, stop=False)

            # ---- tail: last combines, evict, project, scale, store ----
            # Per d-chunk: close the po[c] accumulation, evict it, and
            # let its projection matmuls run while the next chunk
            # combines - keeps the PE busy through the whole tail.
            recip_v = aux[:, 1024:1032].bitcast(F32)     # [128, 4]
            po2 = [psum.tile([P, D], F32, name=f"po2_{mt}", tag="ps")
                   for mt in range(MC)]
            for c in range(DC):
                for t in (NT - 2, NT - 1):
                    nc.tensor.matmul(
                        po[c][:],
                        lhsT=pvall[:, t * D + c * P:t * D + (c + 1) * P],
                        rhs=gmat[:, t * M:(t + 1) * M],
                        start=False, stop=(t == NT - 1),
                    )
                if c % 2 == 0:
                    nc.vector.tensor_scalar_add(
                        out=segsb[:, c * M:(c + 1) * M], in0=po[c][:],
                        scalar1=0.0)
                else:
                    nc.scalar.copy(segsb[:, c * M:(c + 1) * M], po[c][:])
                for mt in range(MC):
                    nc.tensor.matmul(
                        po2[mt][:],
                        lhsT=segsb[:, c * M + mt * P:c * M + (mt + 1) * P],
                        rhs=w1t[:, c * D:(c + 1) * D],
                        start=(c == 0), stop=False,
                    )
            outv = out.ap().rearrange("(mt p) d -> p mt d", p=P)
            oall = outall[:].rearrange("p (mt d) -> p mt d", mt=MC)
            for mt in range(MC):
                nc.tensor.matmul(
                    po2[mt][:],
                    lhsT=aux[0:64, mt * P:(mt + 1) * P],
                    rhs=aux[0:64, 512:1024],
                    start=False, stop=True,
                )
                # fold 1/count back in (gmat is exact-fp8 +-1/0, so the
                # per-m scale rides here as a per-partition scalar)
                nc.vector.tensor_scalar(
                    out=outall[:, mt * D:(mt + 1) * D],
                    in0=po2[mt][:],
                    scalar1=recip_v[:, mt:mt + 1],
                    scalar2=None,
                    op0=mult,
                )
                ring = nc.sync if mt % 2 == 0 else nc.scalar
                ring.dma_start(outv[:, mt:mt + 1], oall[:, mt:mt + 1])

    nc.compile()
    return nc


def _fourier_features(pos, dim):
    half = dim // 2
    freqs = np.exp(np.linspace(0.0, math.log(1000.0), half))
    ang = pos[..., None] * freqs
    return np.concatenate([np.sin(ang), np.cos(ang)], axis=-1)


def _host_prep(frame_emb, beat_bounds, W, b, SB):
    JS = SB // P
    NSB = T // SB
    NT = NSB
    NIDX = NSB * JS * S

    s_all = np.clip(beat_bounds[:, :, 0], 0, T - 1).astype(np.int64)
    e_all = np.maximum(
        s_all + 1, np.minimum(beat_bounds[:, :, 1], T)).astype(np.int64)
    recip_all = (1.0 / (e_all - s_all)).astype(np.float32)

    pos = np.clip(np.arange(M, dtype=np.float64) / max(1, M - 1), 0.0, 1.0)
    ff = _fourier_features(pos, POS_DIM)                  # [M, 32]

    w1p = np.ascontiguousarray(
        W[:D, :].astype(np.float16).reshape(DC, P, D)
        .transpose(1, 0, 2).reshape(P, DC * D))
    f8 = mybir.dt.np(mybir.dt.float8e4)

    in_maps = []
    for i in range(B):
        s, e, recip = s_all[i], e_all[i], recip_all[i]
        count = (e - s).astype(np.float32)

        # aux: count-scaled fourier columns (so the bias term rides the
        # projection accumulation), W2/b rows, and the 1/count scales
        aux = np.zeros((P, 1032), dtype=np.float16)
        aux[0:POS_DIM, 0:M] = (ff.T * count[None, :]).astype(np.float16)
        aux[POS_DIM, 0:M] = count.astype(np.float16)
        aux[0:POS_DIM, 512:512 + D] = W[D:D + POS_DIM, :].astype(np.float16)
        aux[POS_DIM, 512:512 + D] = b.astype(np.float16)
        aux[:, 1024:1032] = np.ascontiguousarray(
            recip.reshape(MC, P).T, dtype=np.float32).view(np.float16)
        allpos = np.concatenate([(s - 1)[s > 0], e - 1])
        idxv = np.full(NIDX, -1.0, dtype=np.float32)
        slotmap = {}
        for k in range(NSB):
            offs = np.unique(allpos[(allpos // SB) == k] % SB)
            if len(offs) > S - 1:
                raise OverflowError(
                    f"superblock {k}: {len(offs)} boundaries > {S - 1}")
            base = k * JS * S
            for jj in range(JS):
                idxv[base + jj * S] = (SB - 1 - jj) // JS   # sum slot
                idxv[base + jj * S + 1:base + jj * S + 1 + len(offs)] = (
                    np.where(offs >= jj, (offs - jj) // JS, -1.0))
            for j, o in enumerate(offs):
                slotmap[(k, int(o))] = j + 1

        # stationary masks: uslots[p, c] = (idxv[c] >= p)
        us = (idxv[None, :] >= np.arange(P, dtype=np.float32)[:, None])

        # G[slot, m]: +1 at e-boundary slot, -1 at s-boundary slot,
        # +1 on the sum slots of fully-spanned superblocks (exact in
        # fp8; the 1/count scale is applied after projection)
        gm = np.zeros((NSB * S, M), dtype=np.float32)
        for m in range(M):
            pe = int(e[m]) - 1
            ke = pe // SB
            gm[ke * S + slotmap[(ke, pe % SB)], m] += 1.0
            ks = 0
            if s[m] > 0:
                ps = int(s[m]) - 1
                ks = ps // SB
                gm[ks * S + slotmap[(ks, ps % SB)], m] -= 1.0
            for k in range(ks, ke):
                gm[k * S, m] += 1.0
        gmat = np.ascontiguousarray(
            gm.reshape(NT, P, M).transpose(1, 0, 2)
            .reshape(P, NT * M)).astype(f8)

        in_maps.append({
            "frame": np.ascontiguousarray(frame_emb[i], dtype=np.float16),
            "uslots": us.astype(f8),
            "gmat": gmat,
            "w1p": w1p,
            "aux": aux,
        })
    return in_maps


def get_nc(SB=512):
    if SB not in _CACHED_NC:
        _CACHED_NC[SB] = _build_nc(SB)
    return _CACHED_NC[SB]


def kernel(frame_emb, beat_bounds, W, b, _trace=False):
    frame_emb = np.asarray(frame_emb)
    beat_bounds = np.asarray(beat_bounds)
    W = np.asarray(W)
    b = np.asarray(b)
    in_maps = None
    for SB in (512, 256, 128):
        try:
            in_maps = _host_prep(frame_emb, beat_bounds, W, b, SB)
            break
        except OverflowError:
            continue
    if in_maps is None:
        raise RuntimeError("too many segment boundaries per superblock")
    nc = get_nc(SB)
    res = bass_utils.run_bass_kernel_spmd(
        nc, in_maps, core_ids=list(range(N_CORES)), trace=_trace)
    out = np.stack([res.results[i]["out"] for i in range(B)],
                   axis=0).astype(np.float32)
    if _trace:
        kernel.last_results = res
    return out
